# revision 1
# baseline (speedup 1.0000x reference)
"""Fused MHA-with-RoPE kernel for one TRN2 chip (8 NeuronCores).

Sharding: core c handles batch b = c//2 and head-group g = c%2 (8 of 16
heads).  Each core:
  phase 1: QKV projections (fp32r matmuls) + RoPE on q/k, q/k/v spilled to
           DRAM in attention-friendly layouts (qT/kT transposed, v natural)
  phase 2: causal attention per head, computed transposed (sT[j,i]) so no
           P transposes are needed; softmax denominator via ones-matmul;
           normalization via PE ones-broadcast + DVE multiply
  phase 3: output projection partial = av @ WoT over this core's 1024 dims,
           per-512-row chunks with pipelined pair-ReduceScatter (issued on
           the Vector queue so gpsimd DMA issue is not blocked)
Host: shards inputs into partition-tiled layouts, reassembles the
RS-interleaved rows.

Self-contained: only numpy + concourse (runtime libs) + the axon boot shim.
"""

import math
import os
import sys
import types
from contextlib import ExitStack

import numpy as np

import concourse.bass as bass
import concourse.tile as tile
from concourse import bacc, mybir
from concourse.bass_utils import run_bass_kernel_spmd

# ---------------------------------------------------------------- constants
B, S, D = 4, 2048, 2048
H, HD = 16, 128
GROUPS = 2            # head groups (cores per batch)
HLOC = H // GROUPS    # heads per core = 8
E = HLOC * HD         # local qkv width = 1024
N_CORES = 8
CORE_IDS = list(range(N_CORES))
SCALE = 1.0 / math.sqrt(HD)
NEG = -1.0e30
ROPE_BASE = 10000.0

F32 = mybir.dt.float32
F32R = mybir.dt.float32r
BF16 = mybir.dt.bfloat16

_cache = {}


def _register_ntff_hook():
    """trn_boot can't register the NTFF profile hook (antenv.axon_hooks is
    missing from this image); recreate it so BASS_TRACE=1 profiling works."""
    if "antenv.axon_hooks" in sys.modules:
        return
    try:
        from trn_agent_boot.trn_boot import _ntff_profile_via_ctypes

        holder = {"h": _ntff_profile_via_ctypes("/opt/axon/libaxon_pjrt.so")}
        mod = types.ModuleType("antenv.axon_hooks")
        mod.get_axon_ntff_profile_hook = lambda: holder["h"]
        mod.set_axon_ntff_profile_hook = lambda h: holder.__setitem__("h", h)
        sys.modules["antenv.axon_hooks"] = mod
    except Exception:
        pass


def _host_tables():
    inv_freq = 1.0 / (ROPE_BASE ** (np.arange(0, HD, 2, dtype=np.float64) / HD))
    pos = np.arange(S, dtype=np.float64)
    freqs = pos[:, None] * inv_freq[None, :]
    emb = np.concatenate([freqs, freqs], axis=-1)        # [S, HD]
    cosT = np.ascontiguousarray(np.cos(emb).T.astype(np.float32))  # [HD, S]
    sinT = np.ascontiguousarray(np.sin(emb).T.astype(np.float32))
    sinF = sinT.copy()
    sinF[: HD // 2] *= -1.0                              # fold rotate_half sign
    return cosT, sinF


def _host_masks():
    # masks[j_local, o, i_local]: 0 if i_local >= o*128 + j_local else NEG
    m = np.empty((128, 4, 512), np.float32)
    jj = np.arange(128)[:, None]
    ii = np.arange(512)[None, :]
    for o in range(4):
        m[:, o, :] = np.where(ii >= o * 128 + jj, 0.0, NEG)
    return m


def _build_nc():
    nc = bacc.Bacc("TRN2", target_bir_lowering=False, debug=False,
                   num_devices=N_CORES)

    # host-pre-tiled inputs: partition-contiguous DMA layouts
    xs_e = nc.dram_tensor("xs", [4, 128, 16, 512], F32R, kind="ExternalInput")
    wq_e = nc.dram_tensor("wq", [HLOC, 128, 16, 128], F32R,
                          kind="ExternalInput")
    wk_e = nc.dram_tensor("wk", [HLOC, 128, 16, 128], F32R,
                          kind="ExternalInput")
    wv_e = nc.dram_tensor("wv", [4, 128, 16, 256], F32R, kind="ExternalInput")
    wo_e = nc.dram_tensor("wo", [128, HLOC, D], F32R, kind="ExternalInput")
    out_e = nc.dram_tensor("out", [4, 512 // GROUPS, D], F32,
                           kind="ExternalOutput")

    cosT_d = nc.inline_tensor(_host_tables()[0], name="cosT")
    sinF_d = nc.inline_tensor(_host_tables()[1], name="sinF")
    masks_d = nc.inline_tensor(_host_masks(), name="masks")
    ones_col_d = nc.inline_tensor(np.ones((128, 1), np.float32), name="ones_col")
    ones_row_d = nc.inline_tensor(np.ones((1, 128), np.float32), name="ones_row")

    with tile.TileContext(nc) as tc, ExitStack() as ctx:
        dram = ctx.enter_context(tc.tile_pool(name="dram", bufs=1, space="DRAM"))
        qh_d = [dram.tile([HD, S], F32R, name=f"qh_d{h}") for h in range(HLOC)]
        kh_d = [dram.tile([HD, S], F32R, name=f"kh_d{h}") for h in range(HLOC)]
        vh_d = [dram.tile([128, 16, HD], F32R, name=f"vh_d{h}")
                for h in range(HLOC)]
        part_d = [dram.tile([512, D], BF16, name=f"part_d{c}")
                  for c in range(4)]
        rs_d = [dram.tile([512 // GROUPS, D], BF16, name=f"rs_d{c}")
                for c in range(4)]

        consts = ctx.enter_context(tc.tile_pool(name="consts", bufs=1))
        masks_sb = consts.tile([128, 4, 512], F32)
        ones_col = consts.tile([128, 1], F32R)
        ones_row = consts.tile([1, 128], F32R)
        nc.gpsimd.dma_start(out=masks_sb[:], in_=masks_d[:])
        nc.gpsimd.dma_start(out=ones_col[:], in_=ones_col_d[:])
        nc.gpsimd.dma_start(out=ones_row[:], in_=ones_row_d[:])

        HF = HD // 2

        # ---------------- phase 1: projections ----------------
        with tc.tile_pool(name="xT", bufs=1) as xT_pool, \
             tc.tile_pool(name="tabs", bufs=1) as tabs:
            cos_sb = tabs.tile([HD, S], F32)
            sinF_sb = tabs.tile([HD, S], F32)
            nc.gpsimd.dma_start(out=cos_sb[:], in_=cosT_d[:])
            nc.gpsimd.dma_start(out=sinF_sb[:], in_=sinF_d[:])

            xs = []
            for sb in range(4):
                xt = xT_pool.tile([128, 16, 512], F32R, name=f"xs{sb}")
                nc.sync.dma_start(out=xt[:], in_=xs_e[sb])
                xs.append(xt)

            ps1_ctx = tc.tile_pool(name="ps1", bufs=4, space="PSUM")
            ps1_all = ps1_ctx.__enter__()
            # q/k projections + RoPE, spilled transposed per head [HD, S]
            with tc.tile_pool(name="wqk", bufs=2) as wqk_pool, \
                 tc.tile_pool(name="rope_wk", bufs=4) as rwk, \
                 tc.tile_pool(name="rot_out", bufs=4) as rout:
                ps1 = ps1_all
                for w_e, o_d, pname in ((wq_e, qh_d, "q"), (wk_e, kh_d, "k")):
                    for m in range(HLOC):
                        w_sb = wqk_pool.tile([128, 16, 128], F32R,
                                             name=f"w{pname}{m}", tag="w")
                        nc.gpsimd.dma_start(out=w_sb[:], in_=w_e[m])
                        for sb in range(4):
                            ps = ps1.tile([128, 512], F32, name="ps_qk",
                                          tag="ps_qk")
                            for dt_ in range(16):
                                nc.tensor.matmul(
                                    ps[:], w_sb[:, dt_, :], xs[sb][:, dt_, :],
                                    start=(dt_ == 0), stop=(dt_ == 15))
                            c_sl = cos_sb[:, bass.ts(sb, 512)]
                            s_sl = sinF_sb[:, bass.ts(sb, 512)]
                            sw = rwk.tile([128, 512], F32, name="sw", tag="sw")
                            nc.scalar.copy(sw[0:HF, :], ps[HF:HD, :])
                            nc.scalar.copy(sw[HF:HD, :], ps[0:HF, :])
                            m1 = rwk.tile([128, 512], F32, name="m1", tag="m1")
                            nc.vector.tensor_mul(m1[:], ps[:], c_sl)
                            m2 = rwk.tile([128, 512], F32, name="m2", tag="m2")
                            nc.vector.tensor_mul(m2[:], sw[:], s_sl)
                            rot = rout.tile([128, 512], F32R, name="rot",
                                            tag="rot")
                            nc.vector.tensor_add(rot[:], m1[:], m2[:])
                            nc.gpsimd.dma_start(
                                out=o_d[m][:, bass.ts(sb, 512)], in_=rot[:])

            # v projection, spilled per head partition-tiled [128, 16, HD]
            with tc.tile_pool(name="wv", bufs=2) as wv_pool, \
                 tc.tile_pool(name="vout", bufs=4) as vout:
                ps1b = ps1_all
                for n in range(4):                     # 256-wide = 2 heads
                    wv_sb = wv_pool.tile([128, 16, 256], F32R, name=f"wv{n}",
                                         tag="wv")
                    nc.sync.dma_start(out=wv_sb[:], in_=wv_e[n])
                    for st in range(16):
                        ps = ps1b.tile([128, 256], F32, name="ps_v", tag="ps_v")
                        for dt_ in range(16):
                            nc.tensor.matmul(
                                ps[:], xs[st // 4][:, dt_,
                                                   bass.ts(st % 4, 128)],
                                wv_sb[:, dt_, :],
                                start=(dt_ == 0), stop=(dt_ == 15))
                        vt = vout.tile([128, 256], F32R, name="vt", tag="vt")
                        nc.scalar.copy(vt[:], ps[:])
                        for half in range(2):
                            nc.gpsimd.dma_start(
                                out=vh_d[2 * n + half][:, st, :],
                                in_=vt[:, bass.ts(half, 128)])

            ps1_ctx.__exit__(None, None, None)

        # ---------------- phase 2: attention ----------------
        avT_pool = ctx.enter_context(tc.tile_pool(name="avT", bufs=1))
        avT_sb = avT_pool.tile([128, HLOC, S], F32R)
        wo_pool = ctx.enter_context(tc.tile_pool(name="wo", bufs=1))
        wo_sb = wo_pool.tile([128, HLOC, D], F32R)
        nc.gpsimd.dma_start(out=wo_sb[:], in_=wo_e[:])

        with tc.tile_pool(name="qh", bufs=2) as qh_pool, \
             tc.tile_pool(name="kh", bufs=2) as kh_pool, \
             tc.tile_pool(name="vh", bufs=2) as vh_pool, \
             tc.tile_pool(name="wk2", bufs=3) as wk2, \
             tc.tile_pool(name="out3", bufs=3) as out3, \
             tc.tile_pool(name="ps2", bufs=2, space="PSUM") as ps2, \
             tc.tile_pool(name="psacc", bufs=2, space="PSUM") as psacc, \
             tc.tile_pool(name="ps3", bufs=2, space="PSUM") as ps3:

            def emit_wo_chunk(cb):
                for i4 in range(4):
                    im = cb * 4 + i4
                    for eb in range(4):
                        ps = ps3.tile([128, 512], F32, name="ps_o",
                                      tag="ps_o")
                        for hh in range(HLOC):
                            nc.tensor.matmul(
                                ps[:], avT_sb[:, hh, bass.ts(im, 128)],
                                wo_sb[:, hh, bass.ts(eb, 512)],
                                start=(hh == 0), stop=(hh == HLOC - 1))
                        po = out3.tile([128, 512], BF16, name="po", tag="po")
                        nc.scalar.copy(po[:], ps[:])
                        nc.sync.dma_start(
                            out=part_d[cb][bass.ts(i4, 128), bass.ts(eb, 512)],
                            in_=po[:])
                nc.gpsimd.collective_compute(
                    "ReduceScatter",
                    mybir.AluOpType.add,
                    replica_groups=[[0, 1], [2, 3], [4, 5], [6, 7]],
                    ins=[part_d[cb][:]],
                    outs=[rs_d[cb][:]],
                )

            for h in range(HLOC):
                q_sb = qh_pool.tile([128, S], F32R, name=f"qh{h}", tag="qh")
                nc.sync.dma_start(out=q_sb[:], in_=qh_d[h][:])
                k_sb = kh_pool.tile([128, S], F32R, name=f"kh{h}", tag="kh")
                nc.sync.dma_start(out=k_sb[:], in_=kh_d[h][:])
                v_sb = vh_pool.tile([128, 16, 128], F32R, name=f"vh{h}",
                                    tag="vh")
                nc.sync.dma_start(out=v_sb[:], in_=vh_d[h][:])
                for ib in range(4):
                    nj = 4 * (ib + 1)
                    den_ps = psacc.tile([1, 512], F32, name="den", tag="den",
                                        bufs=2)
                    av_ps = psacc.tile([128, 512], F32, name="av", tag="av",
                                       bufs=2)
                    for jt in range(nj):
                        s_ps = ps2.tile([128, 512], F32, name="s_ps",
                                        tag="s_ps")
                        nc.tensor.matmul(s_ps[:], k_sb[:, bass.ts(jt, 128)],
                                         q_sb[:, bass.ts(ib, 512)],
                                         start=True, stop=True)
                        o_diag = jt - 4 * ib
                        if o_diag >= 0:
                            msk = wk2.tile([128, 512], F32, name="msk",
                                           tag="msk")
                            nc.vector.tensor_add(msk[:], s_ps[:],
                                                 masks_sb[:, o_diag, :])
                            src = msk
                        else:
                            src = s_ps
                        pT = wk2.tile([128, 512], F32R, name="pT", tag="pT")
                        nc.scalar.activation(
                            pT[:], src[:], mybir.ActivationFunctionType.Exp,
                            scale=SCALE)
                        nc.tensor.matmul(den_ps[:], ones_col[:], pT[:],
                                         start=(jt == 0), stop=(jt == nj - 1))
                        nc.tensor.matmul(av_ps[:], v_sb[:, jt, :], pT[:],
                                         start=(jt == 0), stop=(jt == nj - 1))
                    rden = wk2.tile([1, 512], F32R, name="rden", tag="rden",
                                    bufs=2)
                    with nc.allow_low_precision(reason="f32r rounding only"):
                        nc.vector.reciprocal(rden[:], den_ps[:])
                    bc_sb = wk2.tile([128, 512], F32R, name="bc_sb",
                                     tag="bcs", bufs=2)
                    nc.gpsimd.partition_broadcast(bc_sb[:], rden[:])
                    nc.vector.tensor_mul(avT_sb[:, h, bass.ts(ib, 512)],
                                         av_ps[:], bc_sb[:])

            for cb in range(4):
                emit_wo_chunk(cb)
            for c4 in range(4):
                nc.gpsimd.dma_start(out=out_e[c4], in_=rs_d[c4][:])

    nc.compile()
    return nc


def kernel(x, Wq, Wk, Wv, Wo):
    _register_ntff_hook()
    if "nc" not in _cache:
        _cache["nc"] = _build_nc()
    nc = _cache["nc"]

    in_maps = []
    for c in CORE_IDS:
        b, g = c // GROUPS, c % GROUPS
        sl = slice(g * E, (g + 1) * E)
        xT = np.ascontiguousarray(x[b].T)                       # [D, S]
        in_maps.append({
            "xs": np.ascontiguousarray(
                xT.reshape(16, 128, 4, 512).transpose(2, 1, 0, 3)),
            "wq": np.ascontiguousarray(
                Wq[sl, :].T.reshape(16, 128, HLOC, 128).transpose(2, 1, 0, 3)),
            "wk": np.ascontiguousarray(
                Wk[sl, :].T.reshape(16, 128, HLOC, 128).transpose(2, 1, 0, 3)),
            "wv": np.ascontiguousarray(
                Wv[sl, :].T.reshape(16, 128, 4, 256).transpose(2, 1, 0, 3)),
            "wo": np.ascontiguousarray(
                Wo[:, sl].T.reshape(HLOC, 128, D).transpose(1, 0, 2)),
        })

    trace = bool(os.environ.get("BASS_TRACE"))
    res = run_bass_kernel_spmd(nc, in_maps, CORE_IDS, trace=trace)
    kernel.last_exec_time_ns = res.exec_time_ns
    kernel.last_res = res

    out = np.empty((B, S, D), np.float32)
    half = 512 // GROUPS
    for c in CORE_IDS:
        b, g = c // GROUPS, c % GROUPS
        r = res.results[c]["out"]          # [4, 256, D]
        for ch in range(4):
            lo = ch * 512 + g * half
            out[b, lo:lo + half, :] = r[ch]
    return out


kernel.last_exec_time_ns = None



# revision 9
# speedup vs baseline: 1.3091x; 1.3091x over previous
"""Fused MHA-with-RoPE kernel for one TRN2 chip (8 NeuronCores).

Sharding: core c handles batch b = c//2 and head-group g = c%2 (8 of 16
heads).  All matmuls in bf16 (fp32 PSUM accumulate):
  phase 1: v projection first (starts as soon as x lands), spilled per-head
           to DRAM; then q/k projections + RoPE, kept SBUF-resident in
           transposed [hd, S] layout.
  phase 2: causal attention, query-block-outer / head-inner so the output
           projection + pair-ReduceScatter for block ib overlaps attention
           of block ib+1.  Scores computed transposed (sT[k, q]); softmax
           denominator via ones-matmul; diagonal tiles use shrunken
           free-dim subranges (causal); normalization deferred off the
           TensorE critical path (reciprocal_approx_fast + broadcast).
Host: shards/pre-tiles inputs in bf16, reassembles RS-interleaved rows.

Self-contained: only numpy + concourse (runtime libs) + the axon boot shim.
"""

import math
import os
import sys
import types
from contextlib import ExitStack

import numpy as np
import ml_dtypes

import concourse.bass as bass
import concourse.tile as tile
from concourse import bacc, mybir
from concourse.bass_utils import run_bass_kernel_spmd

# ---------------------------------------------------------------- constants
B, S, D = 4, 2048, 2048
H, HD = 16, 128
GROUPS = 2            # head groups (cores per batch)
HLOC = H // GROUPS    # heads per core = 8
E = HLOC * HD         # local qkv width = 1024
N_CORES = 8
CORE_IDS = list(range(N_CORES))
SCALE = 1.0 / math.sqrt(HD)
NEG = -1.0e30
ROPE_BASE = 10000.0

F32 = mybir.dt.float32
BF16 = mybir.dt.bfloat16
BF = ml_dtypes.bfloat16

_cache = {}


def _register_ntff_hook():
    """trn_boot can't register the NTFF profile hook (antenv.axon_hooks is
    missing from this image); recreate it so BASS_TRACE=1 profiling works."""
    if "antenv.axon_hooks" in sys.modules:
        return
    try:
        from trn_agent_boot.trn_boot import _ntff_profile_via_ctypes

        holder = {"h": _ntff_profile_via_ctypes("/opt/axon/libaxon_pjrt.so")}
        mod = types.ModuleType("antenv.axon_hooks")
        mod.get_axon_ntff_profile_hook = lambda: holder["h"]
        mod.set_axon_ntff_profile_hook = lambda h: holder.__setitem__("h", h)
        sys.modules["antenv.axon_hooks"] = mod
    except Exception:
        pass


def _host_tables():
    inv_freq = 1.0 / (ROPE_BASE ** (np.arange(0, HD, 2, dtype=np.float64) / HD))
    pos = np.arange(S, dtype=np.float64)
    freqs = pos[:, None] * inv_freq[None, :]
    emb = np.concatenate([freqs, freqs], axis=-1)        # [S, HD]
    cosT = np.ascontiguousarray(np.cos(emb).T.astype(np.float32))  # [HD, S]
    sinT = np.ascontiguousarray(np.sin(emb).T.astype(np.float32))
    sinF = sinT.copy()
    sinF[: HD // 2] *= -1.0                              # fold rotate_half sign
    return cosT, sinF


def _host_mask():
    # triangular mask for diagonal tiles: keep (j, i) if i >= j else NEG.
    jj = np.arange(128)[:, None]
    ii = np.arange(512)[None, :]
    return np.where(ii >= jj, 0.0, NEG).astype(np.float32)


def _build_nc():
    nc = bacc.Bacc("TRN2", target_bir_lowering=False, debug=False,
                   num_devices=N_CORES)

    # host-pre-tiled bf16 inputs: partition-contiguous DMA layouts
    xs_e = nc.dram_tensor("xs", [4, 128, 16, 512], BF16, kind="ExternalInput")
    wq_e = nc.dram_tensor("wq", [HLOC, 128, 16, 128], BF16,
                          kind="ExternalInput")
    wk_e = nc.dram_tensor("wk", [HLOC, 128, 16, 128], BF16,
                          kind="ExternalInput")
    wv_e = nc.dram_tensor("wv", [128, 16, E], BF16, kind="ExternalInput")
    wo_e = nc.dram_tensor("wo", [128, HLOC, D], BF16, kind="ExternalInput")
    # 8 chunks of 128 rows each (RS halves of 256-row chunks)
    out_e = nc.dram_tensor("out", [8, 128, D], BF16, kind="ExternalOutput")

    cosT_h, sinF_h = _host_tables()
    cosT_d = nc.inline_tensor(cosT_h, name="cosT")
    sinF_d = nc.inline_tensor(sinF_h, name="sinF")
    mask_d = nc.inline_tensor(_host_mask(), name="mask")
    ones_col_d = nc.inline_tensor(np.ones((128, 1), BF), name="ones_col")

    with tile.TileContext(nc) as tc, ExitStack() as ctx:
        dram = ctx.enter_context(tc.tile_pool(name="dram", bufs=1, space="DRAM"))
        vh_d = [dram.tile([128, 16, HD], BF16, name=f"vh_d{h}")
                for h in range(HLOC)]
        part_d = [dram.tile([256, D], BF16, name=f"part_d{c}")
                  for c in range(8)]
        rs_d = [dram.tile([128, D], BF16, name=f"rs_d{c}")
                for c in range(8)]

        consts = ctx.enter_context(tc.tile_pool(name="consts", bufs=1))
        mask_sb = consts.tile([128, 512], F32)
        ones_col = consts.tile([128, 1], BF16)
        nc.gpsimd.dma_start(out=mask_sb[:], in_=mask_d[:])
        nc.gpsimd.dma_start(out=ones_col[:], in_=ones_col_d[:])

        # warm up the collective path so the first real RS isn't ~45us cold
        warm_in = dram.tile([2, 16], BF16, name="warm_in")
        warm_out = dram.tile([1, 16], BF16, name="warm_out")
        nc.gpsimd.collective_compute(
            "ReduceScatter", mybir.AluOpType.add,
            replica_groups=[[0, 1], [2, 3], [4, 5], [6, 7]],
            ins=[warm_in[:]], outs=[warm_out[:]])

        HF = HD // 2

        # persistent SBUF tensors (qT/kT written in phase 1, read in phase 2)
        qk_pool = ctx.enter_context(tc.tile_pool(name="qk", bufs=1))
        qT_sb = qk_pool.tile([128, HLOC, S], BF16)   # 4MB
        kT_sb = qk_pool.tile([128, HLOC, S], BF16)   # 4MB

        # ---------------- phase 1: projections (stream x by s-block) -------
        with tc.tile_pool(name="xT", bufs=2) as xT_pool, \
             tc.tile_pool(name="tabs", bufs=1) as tabs, \
             tc.tile_pool(name="wv", bufs=1) as wv_pool, \
             tc.tile_pool(name="wqk", bufs=3) as wqk_pool, \
             tc.tile_pool(name="vps", bufs=4, space="PSUM") as vps, \
             tc.tile_pool(name="vout", bufs=3) as vout, \
             tc.tile_pool(name="qkps", bufs=4, space="PSUM") as qkps, \
             tc.tile_pool(name="rwk", bufs=4) as rwk:
            cos_sb = tabs.tile([HD, S], F32)
            sinF_sb = tabs.tile([HD, S], F32)
            nc.gpsimd.dma_start(out=cos_sb[:], in_=cosT_d[:])
            nc.gpsimd.dma_start(out=sinF_sb[:], in_=sinF_d[:])

            # per-dt-chunk loads so the first matmuls start within ~2us
            wv_sb = wv_pool.tile([128, 16, E], BF16)
            for sb in range(4):
                xt = xT_pool.tile([128, 16, 512], BF16, name="xs", tag="xs")
                for dt_ in range(16):
                    if sb == 0:
                        nc.sync.dma_start(out=wv_sb[:, dt_, :],
                                          in_=wv_e[:, dt_, :])
                    nc.sync.dma_start(out=xt[:, dt_, :],
                                      in_=xs_e[sb, :, dt_, :])

                # v projection for these 512 rows: x-chunk stationary,
                # 512-wide wv moving; spilled per head [128 keys, 16 st, HD]
                for s4 in range(4):
                    st = sb * 4 + s4
                    for n in range(2):
                        ps = vps.tile([128, 512], F32, name="ps_v", tag="ps_v")
                        for dt_ in range(16):
                            nc.tensor.matmul(
                                ps[:],
                                xt[:, dt_, bass.ts(s4, 128)],
                                wv_sb[:, dt_, bass.ts(n, 512)],
                                start=(dt_ == 0), stop=(dt_ == 15))
                        vt = vout.tile([128, 512], BF16, name="vt", tag="vt")
                        nc.scalar.copy(vt[:], ps[:])
                        for q4 in range(4):
                            nc.gpsimd.dma_start(
                                out=vh_d[4 * n + q4][:, st, :],
                                in_=vt[:, bass.ts(q4, 128)])

                # q/k projections + RoPE for these rows -> resident qT/kT
                for w_e, o_sb, pname in ((wq_e, qT_sb, "q"), (wk_e, kT_sb, "k")):
                    for m in range(HLOC):
                        w_sb = wqk_pool.tile([128, 16, 128], BF16,
                                             name="w", tag="w")
                        nc.sync.dma_start(out=w_sb[:], in_=w_e[m])
                        ps = qkps.tile([128, 512], F32, name="ps_qk",
                                       tag="ps_qk")
                        for dt_ in range(16):
                            nc.tensor.matmul(
                                ps[:], w_sb[:, dt_, :], xt[:, dt_, :],
                                start=(dt_ == 0), stop=(dt_ == 15))
                        c_sl = cos_sb[:, bass.ts(sb, 512)]
                        s_sl = sinF_sb[:, bass.ts(sb, 512)]
                        sw = rwk.tile([128, 512], F32, name="sw", tag="sw")
                        nc.scalar.copy(sw[0:HF, :], ps[HF:HD, :])
                        nc.scalar.copy(sw[HF:HD, :], ps[0:HF, :])
                        m1 = rwk.tile([128, 512], F32, name="m1", tag="m1")
                        nc.vector.tensor_mul(m1[:], ps[:], c_sl)
                        m2 = rwk.tile([128, 512], F32, name="m2", tag="m2")
                        nc.vector.tensor_mul(m2[:], sw[:], s_sl)
                        nc.vector.tensor_add(
                            o_sb[:, m, bass.ts(sb, 512)], m1[:], m2[:])

        # ---------------- phase 2: attention + output proj ----------------
        p2_pool = ctx.enter_context(tc.tile_pool(name="p2", bufs=1))
        # per-block avT tiles avoid false deps between out-proj(ib) reads
        # and attention(ib+1) normalization writes
        avT_ib = [p2_pool.tile([128, HLOC, 512], BF16, name=f"avT{i}")
                  for i in range(4)]
        wo_sb = p2_pool.tile([128, HLOC, D], BF16)   # 4MB
        for hh in range(HLOC):
            nc.scalar.dma_start(out=wo_sb[:, hh, :], in_=wo_e[:, hh, :])

        with tc.tile_pool(name="vh", bufs=3) as vh_pool, \
             tc.tile_pool(name="wk2", bufs=6) as wk2, \
             tc.tile_pool(name="out3", bufs=4) as out3, \
             tc.tile_pool(name="ps2", bufs=2, space="PSUM") as ps2, \
             tc.tile_pool(name="psden", bufs=2, space="PSUM") as psden, \
             tc.tile_pool(name="psav", bufs=2, space="PSUM") as psav, \
             tc.tile_pool(name="ps3", bufs=2, space="PSUM") as ps3:

            def emit_norm(ib, h, den_ps, av_ps):
                # deferred normalization (off the TensorE critical path)
                rden = wk2.tile([1, 512], F32, name="rden", tag="rden")
                nc.vector.reciprocal_approx_fast(rden[:], den_ps[:])
                bc_sb = wk2.tile([128, 512], F32, name="bc_sb", tag="bcs")
                nc.gpsimd.partition_broadcast(bc_sb[:], rden[:])
                nc.vector.tensor_mul(avT_ib[ib][:, h, :], av_ps[:], bc_sb[:])

            for ib in range(4):
                nj = 4 * (ib + 1)
                nst = nj  # v st-chunks needed for this block
                pending = None
                for h in range(HLOC):
                    v_sb = vh_pool.tile([128, 16, HD], BF16,
                                        name="vh", tag="vh")
                    nc.gpsimd.dma_start(out=v_sb[:, 0:nst, :],
                                        in_=vh_d[h][:, 0:nst, :])
                    den_ps = psden.tile([1, 512], F32, name="den", tag="den")
                    av_ps = psav.tile([128, 512], F32, name="av", tag="av")
                    for jt in range(nj):
                        o_diag = jt - 4 * ib
                        n_q = 512 if o_diag < 0 else 512 - 128 * o_diag
                        q0 = 512 - n_q           # first valid q col in block
                        s_ps = ps2.tile([128, 512], F32, name="s_ps",
                                        tag="s_ps")
                        nc.tensor.matmul(
                            s_ps[:, 0:n_q],
                            kT_sb[:, h, bass.ts(jt, 128)],
                            qT_sb[:, h, ib * 512 + q0: (ib + 1) * 512],
                            start=True, stop=True)
                        if o_diag >= 0:
                            msk = wk2.tile([128, 512], F32, name="msk",
                                           tag="msk")
                            nc.vector.tensor_add(msk[:, 0:n_q],
                                                 s_ps[:, 0:n_q],
                                                 mask_sb[:, 0:n_q])
                            src = msk
                        else:
                            src = s_ps
                        pT = wk2.tile([128, 512], BF16, name="pT", tag="pT")
                        nc.scalar.activation(
                            pT[:, 0:n_q], src[:, 0:n_q],
                            mybir.ActivationFunctionType.Exp, scale=SCALE)
                        nc.tensor.matmul(den_ps[:, q0:512], ones_col[:],
                                         pT[:, 0:n_q],
                                         start=(jt == 0), stop=(jt == nj - 1))
                        nc.tensor.matmul(av_ps[:, q0:512], v_sb[:, jt, :],
                                         pT[:, 0:n_q],
                                         start=(jt == 0), stop=(jt == nj - 1))
                    # normalize the PREVIOUS head here so this head's DVE
                    # mask work isn't queued behind the recip/bc chain
                    if pending is not None:
                        emit_norm(*pending)
                    pending = (ib, h, den_ps, av_ps)
                emit_norm(*pending)

                # output projection for this 512-row block, two 256-row
                # RS chunks so the collective pipelines with attention ib+1
                for half in range(2):
                    cb = ib * 2 + half
                    for i2 in range(2):
                        im = half * 2 + i2   # 128-row subtile within block
                        for eb in range(4):
                            ps = ps3.tile([128, 512], F32, name="ps_o",
                                          tag="ps_o")
                            for hh in range(HLOC):
                                nc.tensor.matmul(
                                    ps[:], avT_ib[ib][:, hh, bass.ts(im, 128)],
                                    wo_sb[:, hh, bass.ts(eb, 512)],
                                    start=(hh == 0), stop=(hh == HLOC - 1))
                            po = out3.tile([128, 512], BF16, name="po",
                                           tag="po")
                            nc.scalar.copy(po[:], ps[:])
                            nc.gpsimd.dma_start(
                                out=part_d[cb][bass.ts(i2, 128),
                                               bass.ts(eb, 512)],
                                in_=po[:])
                    nc.gpsimd.collective_compute(
                        "ReduceScatter",
                        mybir.AluOpType.add,
                        replica_groups=[[0, 1], [2, 3], [4, 5], [6, 7]],
                        ins=[part_d[cb][:]],
                        outs=[rs_d[cb][:]],
                    )

            for cb in range(8):
                nc.sync.dma_start(out=out_e[cb], in_=rs_d[cb][:])

    nc.compile()
    return nc


def kernel(x, Wq, Wk, Wv, Wo):
    _register_ntff_hook()
    if "nc" not in _cache:
        _cache["nc"] = _build_nc()
    nc = _cache["nc"]

    in_maps = []
    for c in CORE_IDS:
        b, g = c // GROUPS, c % GROUPS
        sl = slice(g * E, (g + 1) * E)
        xT = np.ascontiguousarray(x[b].T)                       # [D, S]
        in_maps.append({
            "xs": np.ascontiguousarray(
                xT.reshape(16, 128, 4, 512).transpose(2, 1, 0, 3)).astype(BF),
            "wq": np.ascontiguousarray(
                Wq[sl, :].T.reshape(16, 128, HLOC, 128)
                .transpose(2, 1, 0, 3)).astype(BF),
            "wk": np.ascontiguousarray(
                Wk[sl, :].T.reshape(16, 128, HLOC, 128)
                .transpose(2, 1, 0, 3)).astype(BF),
            "wv": np.ascontiguousarray(
                Wv[sl, :].T.reshape(16, 128, E).transpose(1, 0, 2)).astype(BF),
            "wo": np.ascontiguousarray(
                Wo[:, sl].T.reshape(HLOC, 128, D).transpose(1, 0, 2)).astype(BF),
        })

    trace = bool(os.environ.get("BASS_TRACE"))
    res = run_bass_kernel_spmd(nc, in_maps, CORE_IDS, trace=trace)
    kernel.last_exec_time_ns = res.exec_time_ns
    kernel.last_res = res

    out = np.empty((B, S, D), np.float32)
    for c in CORE_IDS:
        b, g = c // GROUPS, c % GROUPS
        r = np.asarray(res.results[c]["out"]).astype(np.float32)  # [8,128,D]
        for ch in range(8):
            lo = ch * 256 + g * 128
            out[b, lo:lo + 128, :] = r[ch]
    return out


kernel.last_exec_time_ns = None
